# revision 11
# baseline (speedup 1.0000x reference)
"""CSPN 3x3 propagation step on 8 Trainium2 NeuronCores.

out[b,0,r,c] = sum_k aff[b,k,r,c] * patch_k(cur)[r,c], with the center tap
(k=4) taken from coarse_seg instead of cur_seg. Zero padding at image edges.

Sharding: pure data parallel over batch (16 images -> 2 per core), one SPMD
Bass program run on all 8 cores with per-core input slices.

Per-core layout (per 512x512 image): rows are packed PARTITION-MAJOR,
r = 4p + t (partition p in 0..127, sub-row t in 0..3).  cur_seg is staged in
a single padded tile tCur[128, 2(img), 6, 514] where slot j holds row
4p-1+j and columns are shifted by one (zero columns at 0 and 513, zero rows
at (p=0,j=0) and (p=127,j=5), memset once).  Every 3x3 tap k=(dy,dx) is then
ONE tensor_mul against the window tCur[:, b, dy+1:dy+5, dx+1:dx+513] -- no
per-edge fixup products.  The 6-row overlapping windows load in one DMA per
image pair with 12KB/partition descriptors plus two single-partition fixups.

All DMA rides the two HWDGE rings (ACT=nc.scalar, SP=nc.sync) -- on real HW
every channel shares the same ~360GB/s DMA-engine pool, so a third (Pool
swdge) channel buys nothing while blocking the Pool ALU; dropping it frees
Pool for products.  Affinity loads in 3-plane batches (one dma_start each,
8KB descriptors) to minimize DGE/semaphore overhead:
  ACT: B0{6,7,8}(b0), B2{3,4,5}(b0), B0(b1), B2(b1), out(b0)
  SP:  tCur trio, B1{0,1,2}(b0), tC(b0), B1(b1), tC(b1), out(b1)
Stores are software-pipelined: image b's store issues after image b+1's
loads, so the rings never gate on compute.

Compute: DVE multiplies planes 6,7,8,0,1,2; Pool multiplies 3,5 and the
center product Osb = aff4*coarse; PE accumulates the 8 non-center products
into PSUM via identity matmuls in float32r; root adds fold PSUM into Osb
(DVE low half, Pool high half); one 1MB store per image.
"""

import sys

import numpy as np

if "/opt/trn_rl_repo" not in sys.path:
    sys.path.insert(0, "/opt/trn_rl_repo")

B_PER_CORE = 2
N_CORES = 8
H = 512
W = 512
NBLK = 4  # sub-rows per partition
WPAD = W + 2  # zero column on each side
NROW = 6  # rows 4p-1 .. 4p+4

_compiled = None
_compiled_reps = {}


def _build_program(reps=1):
    """reps>1 repeats the whole per-core computation `reps` times inside one
    NEFF -- used only to measure device time through the dispatch noise."""
    import concourse.bacc as bacc
    import concourse.mybir as mybir
    import concourse.tile as tile
    from concourse.ap import AP

    fp32 = mybir.dt.float32
    fp32r = mybir.dt.float32r

    nc = bacc.Bacc(
        "TRN2",
        target_bir_lowering=False,
        debug=False,
        enable_asserts=False,
        num_devices=N_CORES,
    )

    aff_t = nc.dram_tensor("affinity", [B_PER_CORE, 9, H, W], fp32, kind="ExternalInput")
    cur_t = nc.dram_tensor("cur_seg", [B_PER_CORE, 1, H, W], fp32, kind="ExternalInput")
    coa_t = nc.dram_tensor("coarse_seg", [B_PER_CORE, 1, H, W], fp32, kind="ExternalInput")
    idn_d = nc.dram_tensor("ident", [128, 128], fp32r, kind="ExternalInput").ap()
    out_d = nc.dram_tensor("out", [B_PER_CORE, 1, H, W], fp32, kind="ExternalOutput").ap()
    aff_d = aff_t.ap()
    coa_d = coa_t.ap()

    IMG = H * W  # elements per image plane

    # tap k=(dy,dx): window tCur[:, dy+1:dy+5, dx+1:dx+513]
    def win(tcur, k):
        dy, dx = k // 3 - 1, k % 3 - 1
        return tcur[:, dy + 1 : dy + 5, dx + 1 : dx + 513]

    with tile.TileContext(nc) as tc:
        with (
            tc.tile_pool(name="idn", bufs=1) as idn_pool,
            tc.tile_pool(name="cur", bufs=2) as cur_pool,
            tc.tile_pool(name="aff", bufs=3) as aff_pool,
            tc.tile_pool(name="coa", bufs=2) as coa_pool,
            tc.tile_pool(name="prod", bufs=2) as prod_pool,
            tc.tile_pool(name="osb", bufs=2) as osb_pool,
            tc.tile_pool(name="ps", bufs=4, space="PSUM") as ps_pool,
        ):
            tI = idn_pool.tile([128, 128], fp32r)
            nc.scalar.dma_start(out=tI[:], in_=idn_d[:])

            # Persistent padded cur tiles (one per image parity), zero
            # borders memset ONCE; per-image DMAs overwrite only data regions.
            tcur_bufs = []
            for par in range(2):
                tCur = cur_pool.tile([128, NROW, WPAD], fp32, name=f"tCur{par}")
                (nc.vector if par else nc.gpsimd).memset(tCur[:], 0.0)
                tcur_bufs.append(tCur)

            pending_store = [None]

            def flush_store():
                if pending_store[0] is not None:
                    ring, dst, src = pending_store[0]
                    ring.dma_start(out=dst, in_=src)
                    pending_store[0] = None

            for r in range(reps):
                for b in range(B_PER_CORE):
                    tCur = tcur_bufs[b % 2]

                    # --- cur_seg trio into the padded tile (slot j = row
                    # 4p-1+j).  All three are full-width multi-partition
                    # loads; the (p=0,j=0) and (p=127,j=5) slots stay zero.
                    # j=1..4 <- rows 4p..4p+3 (all 128 partitions)
                    nc.sync.dma_start(
                        out=tCur[:, 1:5, 1 : W + 1],
                        in_=AP(cur_t, b * IMG, [[4 * W, 128], [W, 4], [1, W]]),
                    )
                    # j=5 <- row 4p+4 (p=0..126)
                    nc.sync.dma_start(
                        out=tCur[0:127, 5, 1 : W + 1],
                        in_=AP(cur_t, b * IMG + 4 * W, [[4 * W, 127], [1, W]]),
                    )
                    # j=0 <- row 4p-1 (p=1..127)
                    nc.sync.dma_start(
                        out=tCur[1:128, 0, 1 : W + 1],
                        in_=AP(cur_t, b * IMG + 3 * W, [[4 * W, 127], [1, W]]),
                    )
                    # --- affinity batches: 3 planes per dma_start.
                    def batch(k0, ring):
                        t = aff_pool.tile([128, 3, NBLK, W], fp32, tag="aff")
                        ring.dma_start(
                            out=t[:],
                            in_=AP(
                                aff_t,
                                (b * 9 + k0) * IMG,
                                [[NBLK * W, 128], [IMG, 3], [W, NBLK], [1, W]],
                            ),
                        )
                        return t

                    B0 = batch(6, nc.scalar)  # planes 6,7,8
                    B1 = batch(0, nc.sync)    # planes 0,1,2
                    B2 = batch(3, nc.scalar)  # planes 3,4,5
                    tC = coa_pool.tile([128, NBLK, W], fp32, tag="coa")
                    nc.sync.dma_start(
                        out=tC[:],
                        in_=coa_d[b, 0].rearrange("(p t) c -> p t c", p=128),
                    )

                    # previous image's store rides after this image's loads
                    flush_store()

                    # PSUM accumulators (2 banks each)
                    ps_lo = ps_pool.tile([128, 2, W], fp32, tag="ps")
                    ps_hi = ps_pool.tile([128, 2, W], fp32, tag="ps")

                    def accum(P, start, stop):
                        for t in range(NBLK):
                            pst = ps_lo[:, t, :] if t < 2 else ps_hi[:, t - 2, :]
                            nc.tensor.matmul(pst, tI[:], P[:, t, :], start=start, stop=stop)

                    # --- products: one mul per tap.
                    for i, (tile_, ki, k) in enumerate(
                        [
                            (B0, 0, 6), (B0, 1, 7), (B0, 2, 8),
                            (B1, 0, 0), (B1, 1, 1), (B1, 2, 2),
                        ]
                    ):
                        P = prod_pool.tile([128, NBLK, W], fp32r, tag="prod")
                        nc.vector.tensor_mul(out=P[:], in0=tile_[:, ki], in1=win(tCur, k))
                        accum(P, start=(i == 0), stop=False)
                    P3 = prod_pool.tile([128, NBLK, W], fp32r, tag="prod")
                    nc.gpsimd.tensor_mul(out=P3[:], in0=B2[:, 0], in1=win(tCur, 3))
                    accum(P3, start=False, stop=False)
                    P5 = prod_pool.tile([128, NBLK, W], fp32r, tag="prod")
                    nc.gpsimd.tensor_mul(out=P5[:], in0=B2[:, 2], in1=win(tCur, 5))
                    accum(P5, start=False, stop=True)

                    # --- root: Osb = aff4*coarse + psum
                    Osb = osb_pool.tile([128, NBLK, W], fp32, tag="osb")
                    nc.gpsimd.tensor_mul(out=Osb[:], in0=B2[:, 1], in1=tC[:])
                    nc.vector.tensor_add(out=Osb[:, 0:2, :], in0=Osb[:, 0:2, :], in1=ps_lo[:])
                    nc.vector.tensor_add(out=Osb[:, 2:4, :], in0=Osb[:, 2:4, :], in1=ps_hi[:])

                    out_rows = out_d[b, 0].rearrange("(p t) c -> p t c", p=128)
                    pending_store[0] = (
                        nc.scalar if b == 0 else nc.sync,
                        out_rows[:],
                        Osb[:],
                    )

            flush_store()

    nc.compile()
    return nc


def _get_program(reps=1):
    global _compiled
    if reps != 1:
        if reps not in _compiled_reps:
            _compiled_reps[reps] = _build_program(reps)
        return _compiled_reps[reps]
    if _compiled is None:
        _compiled = _build_program()
    return _compiled


def _in_maps(affinity, cur_seg, coarse_seg):
    ident = np.eye(128, dtype=np.float32)
    maps = []
    for j in range(N_CORES):
        s = slice(j * B_PER_CORE, (j + 1) * B_PER_CORE)
        maps.append(
            {
                "affinity": np.ascontiguousarray(affinity[s]),
                "cur_seg": np.ascontiguousarray(cur_seg[s]),
                "coarse_seg": np.ascontiguousarray(coarse_seg[s]),
                "ident": ident,
            }
        )
    return maps


def kernel(affinity, cur_seg, coarse_seg, i=None, **_unused):
    from concourse.bass_utils import run_bass_kernel_spmd

    nc = _get_program()

    affinity = np.ascontiguousarray(affinity, dtype=np.float32)
    cur_seg = np.ascontiguousarray(cur_seg, dtype=np.float32)
    coarse_seg = np.ascontiguousarray(coarse_seg, dtype=np.float32)

    res = run_bass_kernel_spmd(
        nc, _in_maps(affinity, cur_seg, coarse_seg), core_ids=list(range(N_CORES))
    )
    out = np.concatenate([r["out"] for r in res.results], axis=0)
    return out


# revision 16
# speedup vs baseline: 1.4785x; 1.4785x over previous
"""CSPN 3x3 propagation step on 8 Trainium2 NeuronCores.

out[b,0,r,c] = sum_k aff[b,k,r,c] * patch_k(cur)[r,c], with the center tap
(k=4) taken from coarse_seg instead of cur_seg. Zero padding at image edges.

Sharding: pure data parallel over batch (16 images -> 2 per core), one SPMD
Bass program run on all 8 cores with per-core input slices.

Per-core layout (per 512x512 image): rows are packed PARTITION-MAJOR,
r = 4p + t (partition p in 0..127, sub-row t in 0..3).  cur_seg is staged in
a single padded tile tCur[128, 2(img), 6, 514] where slot j holds row
4p-1+j and columns are shifted by one (zero columns at 0 and 513, zero rows
at (p=0,j=0) and (p=127,j=5), memset once).  Every 3x3 tap k=(dy,dx) is then
ONE tensor_mul against the window tCur[:, b, dy+1:dy+5, dx+1:dx+513] -- no
per-edge fixup products.  The 6-row overlapping windows load in one DMA per
image pair with 12KB/partition descriptors plus two single-partition fixups.

All DMA rides the two HWDGE rings (ACT=nc.scalar, SP=nc.sync) -- on real HW
every channel shares the same ~360GB/s DMA-engine pool, so a third (Pool
swdge) channel buys nothing while blocking the Pool ALU; dropping it frees
Pool for products.  Affinity loads in 3-plane batches (one dma_start each,
8KB descriptors) to minimize DGE/semaphore overhead:
  ACT: B0{6,7,8}(b0), B2{3,4,5}(b0), B0(b1), B2(b1), out(b0)
  SP:  tCur trio, B1{0,1,2}(b0), tC(b0), B1(b1), tC(b1), out(b1)
Stores are software-pipelined: image b's store issues after image b+1's
loads, so the rings never gate on compute.

Compute: DVE multiplies planes 6,7,8,0,1,2; Pool multiplies 3,5 and the
center product Osb = aff4*coarse; PE accumulates the 8 non-center products
into PSUM via identity matmuls in float32r; root adds fold PSUM into Osb
(DVE low half, Pool high half); one 1MB store per image.
"""

import sys

import numpy as np

if "/opt/trn_rl_repo" not in sys.path:
    sys.path.insert(0, "/opt/trn_rl_repo")

B_PER_CORE = 2
N_CORES = 8
H = 512
W = 512
NBLK = 4  # sub-rows per partition
WPAD = W + 2  # zero column on each side
NROW = 6  # rows 4p-1 .. 4p+4

_compiled = None
_compiled_reps = {}


def _build_program(reps=1):
    """reps>1 repeats the whole per-core computation `reps` times inside one
    NEFF -- used only to measure device time through the dispatch noise."""
    import concourse.bacc as bacc
    import concourse.mybir as mybir
    import concourse.tile as tile
    from concourse.ap import AP

    fp32 = mybir.dt.float32
    fp32r = mybir.dt.float32r

    nc = bacc.Bacc(
        "TRN2",
        target_bir_lowering=False,
        debug=False,
        enable_asserts=False,
        num_devices=N_CORES,
    )

    aff_t = nc.dram_tensor("affinity", [B_PER_CORE, 9, H, W], fp32, kind="ExternalInput")
    cur_t = nc.dram_tensor("cur_seg", [B_PER_CORE, 1, H, W], fp32, kind="ExternalInput")
    coa_t = nc.dram_tensor("coarse_seg", [B_PER_CORE, 1, H, W], fp32, kind="ExternalInput")
    idn_d = nc.dram_tensor("ident", [128, 128], fp32r, kind="ExternalInput").ap()
    out_d = nc.dram_tensor("out", [B_PER_CORE, 1, H, W], fp32, kind="ExternalOutput").ap()
    aff_d = aff_t.ap()
    coa_d = coa_t.ap()

    IMG = H * W  # elements per image plane
    FLAT = NROW * W + 2  # flat row-major 6x512 + one pad element each end

    # tap k=(dy,dx): rows j=dy+1..dy+4 of the flat tile, shifted dx columns.
    # dx=+-1 windows bleed one element across row boundaries; the resulting
    # garbage edge column of the product is memset to zero before the PE
    # accumulates it (the true zero-padded contribution).
    def win(tcur, k):
        dy, dx = k // 3 - 1, k % 3 - 1
        off = 1 + (dy + 1) * W + dx
        return tcur[:, off : off + NBLK * W].rearrange("p (t c) -> p t c", c=W)

    with tile.TileContext(nc) as tc:
        with (
            tc.tile_pool(name="idn", bufs=1) as idn_pool,
            tc.tile_pool(name="cur", bufs=2) as cur_pool,
            tc.tile_pool(name="aff", bufs=3) as aff_pool,
            tc.tile_pool(name="coa", bufs=2) as coa_pool,
            tc.tile_pool(name="prod", bufs=2) as prod_pool,
            tc.tile_pool(name="osb", bufs=2) as osb_pool,
            tc.tile_pool(name="ps", bufs=4, space="PSUM") as ps_pool,
        ):
            tI = idn_pool.tile([128, 128], fp32r)
            nc.scalar.dma_start(out=tI[:], in_=idn_d[:])

            # Persistent padded cur tiles (one per image parity), zero
            # borders memset ONCE; per-image DMAs overwrite only data regions.
            tcur_bufs = []
            for par in range(2):
                tCur = cur_pool.tile([128, FLAT], fp32, name=f"tCur{par}")
                (nc.vector if par else nc.gpsimd).memset(tCur[:], 0.0)
                tcur_bufs.append(tCur)

            pending_store = [None]

            def flush_store():
                if pending_store[0] is not None:
                    ring, dst, src = pending_store[0]
                    ring.dma_start(out=dst, in_=src)
                    pending_store[0] = None

            for r in range(reps):
                for b in range(B_PER_CORE):
                    tCur = tcur_bufs[b % 2]

                    # --- cur_seg trio: overlapping 6-row windows, fully
                    # contiguous on BOTH sides (12KB descriptors).  Flat slot
                    # j (at offset 1+j*W) holds row 4p-1+j; the (p=0,j=0)
                    # and (p=127,j=5) rows stay zero from the initial memset.
                    # p=1..126: j=0..5 <- rows 4p-1..4p+4
                    nc.sync.dma_start(
                        out=tCur[1:127, 1 : 1 + NROW * W],
                        in_=AP(cur_t, b * IMG + 3 * W, [[4 * W, 126], [1, NROW * W]]),
                    )
                    # p=0: j=1..5 <- rows 0..4
                    nc.sync.dma_start(
                        out=tCur[0:1, 1 + W : 1 + NROW * W],
                        in_=AP(cur_t, b * IMG, [[4 * W, 1], [1, 5 * W]]),
                    )
                    # p=127: j=0..4 <- rows 507..511
                    nc.sync.dma_start(
                        out=tCur[127:128, 1 : 1 + 5 * W],
                        in_=AP(cur_t, b * IMG + 507 * W, [[4 * W, 1], [1, 5 * W]]),
                    )
                    # --- affinity batches: 3 planes per dma_start.
                    def batch(k0, ring):
                        t = aff_pool.tile([128, 3, NBLK, W], fp32, tag="aff")
                        ring.dma_start(
                            out=t[:],
                            in_=AP(
                                aff_t,
                                (b * 9 + k0) * IMG,
                                [[NBLK * W, 128], [IMG, 3], [W, NBLK], [1, W]],
                            ),
                        )
                        return t

                    B0 = batch(6, nc.scalar)  # planes 6,7,8
                    B1 = batch(0, nc.sync)    # planes 0,1,2
                    B2 = batch(3, nc.scalar)  # planes 3,4,5
                    tC = coa_pool.tile([128, NBLK, W], fp32, tag="coa")
                    nc.sync.dma_start(
                        out=tC[:],
                        in_=coa_d[b, 0].rearrange("(p t) c -> p t c", p=128),
                    )

                    # previous image's store rides after this image's loads
                    flush_store()

                    # PSUM accumulators (2 banks each)
                    ps_lo = ps_pool.tile([128, 2, W], fp32, tag="ps")
                    ps_hi = ps_pool.tile([128, 2, W], fp32, tag="ps")

                    def accum(P, start, stop):
                        for t in range(NBLK):
                            pst = ps_lo[:, t, :] if t < 2 else ps_hi[:, t - 2, :]
                            nc.tensor.matmul(pst, tI[:], P[:, t, :], start=start, stop=stop)

                    # --- products: one mul per tap; dx!=0 taps zero their
                    # bled edge column before the PE reads the product.
                    def edge_fix(eng, P, k):
                        dx = k % 3 - 1
                        if dx == -1:
                            eng.memset(P[:, :, 0:1].bitcast(fp32), 0.0)
                        elif dx == 1:
                            eng.memset(P[:, :, W - 1 : W].bitcast(fp32), 0.0)

                    for i, (tile_, ki, k) in enumerate(
                        [
                            (B0, 0, 6), (B0, 1, 7), (B0, 2, 8),
                            (B1, 0, 0), (B1, 1, 1), (B1, 2, 2),
                        ]
                    ):
                        P = prod_pool.tile([128, NBLK, W], fp32r, tag="prod")
                        nc.vector.tensor_mul(out=P[:], in0=tile_[:, ki], in1=win(tCur, k))
                        edge_fix(nc.vector, P, k)
                        accum(P, start=(i == 0), stop=False)
                    P3 = prod_pool.tile([128, NBLK, W], fp32r, tag="prod")
                    nc.gpsimd.tensor_mul(out=P3[:], in0=B2[:, 0], in1=win(tCur, 3))
                    edge_fix(nc.gpsimd, P3, 3)
                    accum(P3, start=False, stop=False)
                    P5 = prod_pool.tile([128, NBLK, W], fp32r, tag="prod")
                    nc.gpsimd.tensor_mul(out=P5[:], in0=B2[:, 2], in1=win(tCur, 5))
                    edge_fix(nc.gpsimd, P5, 5)
                    accum(P5, start=False, stop=True)

                    # --- root: Osb = aff4*coarse + psum
                    Osb = osb_pool.tile([128, NBLK, W], fp32, tag="osb")
                    nc.gpsimd.tensor_mul(out=Osb[:], in0=B2[:, 1], in1=tC[:])
                    nc.vector.tensor_add(out=Osb[:, 0:2, :], in0=Osb[:, 0:2, :], in1=ps_lo[:])
                    nc.vector.tensor_add(out=Osb[:, 2:4, :], in0=Osb[:, 2:4, :], in1=ps_hi[:])

                    out_rows = out_d[b, 0].rearrange("(p t) c -> p t c", p=128)
                    pending_store[0] = (
                        nc.scalar if b == 0 else nc.sync,
                        out_rows[:],
                        Osb[:],
                    )

            flush_store()

    nc.compile()
    return nc


def _get_program(reps=1):
    global _compiled
    if reps != 1:
        if reps not in _compiled_reps:
            _compiled_reps[reps] = _build_program(reps)
        return _compiled_reps[reps]
    if _compiled is None:
        _compiled = _build_program()
    return _compiled


def _in_maps(affinity, cur_seg, coarse_seg):
    ident = np.eye(128, dtype=np.float32)
    maps = []
    for j in range(N_CORES):
        s = slice(j * B_PER_CORE, (j + 1) * B_PER_CORE)
        maps.append(
            {
                "affinity": np.ascontiguousarray(affinity[s]),
                "cur_seg": np.ascontiguousarray(cur_seg[s]),
                "coarse_seg": np.ascontiguousarray(coarse_seg[s]),
                "ident": ident,
            }
        )
    return maps


def kernel(affinity, cur_seg, coarse_seg, i=None, **_unused):
    from concourse.bass_utils import run_bass_kernel_spmd

    nc = _get_program()

    affinity = np.ascontiguousarray(affinity, dtype=np.float32)
    cur_seg = np.ascontiguousarray(cur_seg, dtype=np.float32)
    coarse_seg = np.ascontiguousarray(coarse_seg, dtype=np.float32)

    res = run_bass_kernel_spmd(
        nc, _in_maps(affinity, cur_seg, coarse_seg), core_ids=list(range(N_CORES))
    )
    out = np.concatenate([r["out"] for r in res.results], axis=0)
    return out


# revision 17
# speedup vs baseline: 54.3430x; 36.7561x over previous
"""CSPN 3x3 propagation step on 8 Trainium2 NeuronCores.

out[b,0,r,c] = sum_k aff[b,k,r,c] * patch_k(cur)[r,c], with the center tap
(k=4) taken from coarse_seg instead of cur_seg. Zero padding at image edges.

Sharding: pure data parallel over batch (16 images -> 2 per core), one SPMD
Bass program run on all 8 cores with per-core input slices.

Per-core layout (per 512x512 image): rows are packed PARTITION-MAJOR,
r = 4p + t (partition p in 0..127, sub-row t in 0..3).  cur_seg is staged in
a single padded tile tCur[128, 2(img), 6, 514] where slot j holds row
4p-1+j and columns are shifted by one (zero columns at 0 and 513, zero rows
at (p=0,j=0) and (p=127,j=5), memset once).  Every 3x3 tap k=(dy,dx) is then
ONE tensor_mul against the window tCur[:, b, dy+1:dy+5, dx+1:dx+513] -- no
per-edge fixup products.  The 6-row overlapping windows load in one DMA per
image pair with 12KB/partition descriptors plus two single-partition fixups.

All DMA rides the two HWDGE rings (ACT=nc.scalar, SP=nc.sync) -- on real HW
every channel shares the same ~360GB/s DMA-engine pool, so a third (Pool
swdge) channel buys nothing while blocking the Pool ALU; dropping it frees
Pool for products.  Affinity loads in 3-plane batches (one dma_start each,
8KB descriptors) to minimize DGE/semaphore overhead:
  ACT: B0{6,7,8}(b0), B2{3,4,5}(b0), B0(b1), B2(b1), out(b0)
  SP:  tCur trio, B1{0,1,2}(b0), tC(b0), B1(b1), tC(b1), out(b1)
Stores are software-pipelined: image b's store issues after image b+1's
loads, so the rings never gate on compute.

Compute: DVE multiplies planes 6,7,8,0,1,2; Pool multiplies 3,5 and the
center product Osb = aff4*coarse; PE accumulates the 8 non-center products
into PSUM via identity matmuls in float32r; root adds fold PSUM into Osb
(DVE low half, Pool high half); one 1MB store per image.
"""

import sys

import numpy as np

if "/opt/trn_rl_repo" not in sys.path:
    sys.path.insert(0, "/opt/trn_rl_repo")

B_PER_CORE = 2
N_CORES = 8
H = 512
W = 512
NBLK = 4  # sub-rows per partition
WPAD = W + 2  # zero column on each side
NROW = 6  # rows 4p-1 .. 4p+4

_compiled = None
_compiled_reps = {}


def _build_program(reps=1):
    """reps>1 repeats the whole per-core computation `reps` times inside one
    NEFF -- used only to measure device time through the dispatch noise."""
    import concourse.bacc as bacc
    import concourse.mybir as mybir
    import concourse.tile as tile
    from concourse.ap import AP

    fp32 = mybir.dt.float32
    fp32r = mybir.dt.float32r

    nc = bacc.Bacc(
        "TRN2",
        target_bir_lowering=False,
        debug=False,
        enable_asserts=False,
        num_devices=N_CORES,
    )

    aff_t = nc.dram_tensor("affinity", [B_PER_CORE, 9, H, W], fp32, kind="ExternalInput")
    cur_t = nc.dram_tensor("cur_seg", [B_PER_CORE, 1, H, W], fp32, kind="ExternalInput")
    coa_t = nc.dram_tensor("coarse_seg", [B_PER_CORE, 1, H, W], fp32, kind="ExternalInput")
    idn_d = nc.dram_tensor("ident", [128, 128], fp32r, kind="ExternalInput").ap()
    out_d = nc.dram_tensor("out", [B_PER_CORE, 1, H, W], fp32, kind="ExternalOutput").ap()
    aff_d = aff_t.ap()
    coa_d = coa_t.ap()

    IMG = H * W  # elements per image plane
    FLAT = NROW * W + 2  # flat row-major 6x512 + one pad element each end

    # tap k=(dy,dx): rows j=dy+1..dy+4 of the flat tile, shifted dx columns.
    # dx=+-1 windows bleed one element across row boundaries; the resulting
    # garbage edge column of the product is memset to zero before the PE
    # accumulates it (the true zero-padded contribution).
    def win(tcur, k):
        dy, dx = k // 3 - 1, k % 3 - 1
        off = 1 + (dy + 1) * W + dx
        return tcur[:, off : off + NBLK * W].rearrange("p (t c) -> p t c", c=W)

    with tile.TileContext(nc) as tc:
        with (
            tc.tile_pool(name="idn", bufs=1) as idn_pool,
            tc.tile_pool(name="cur", bufs=2) as cur_pool,
            tc.tile_pool(name="aff", bufs=3) as aff_pool,
            tc.tile_pool(name="coa", bufs=2) as coa_pool,
            tc.tile_pool(name="prod", bufs=2) as prod_pool,
            tc.tile_pool(name="osb", bufs=2) as osb_pool,
            tc.tile_pool(name="ps", bufs=4, space="PSUM") as ps_pool,
        ):
            tI = idn_pool.tile([128, 128], fp32r)
            nc.scalar.dma_start(out=tI[:], in_=idn_d[:])

            # Persistent padded cur tiles (one per image parity), zero
            # borders memset ONCE; per-image DMAs overwrite only data regions.
            tcur_bufs = []
            for par in range(2):
                tCur = cur_pool.tile([128, FLAT], fp32, name=f"tCur{par}")
                (nc.vector if par else nc.gpsimd).memset(tCur[:], 0.0)
                tcur_bufs.append(tCur)

            pending_store = [None]

            def flush_store():
                if pending_store[0] is not None:
                    ring, dst, src = pending_store[0]
                    ring.dma_start(out=dst, in_=src)
                    pending_store[0] = None

            for r in range(reps):
                for b in range(B_PER_CORE):
                    tCur = tcur_bufs[b % 2]

                    # --- cur_seg trio: overlapping 6-row windows, fully
                    # contiguous on BOTH sides (12KB descriptors).  Flat slot
                    # j (at offset 1+j*W) holds row 4p-1+j; the (p=0,j=0)
                    # and (p=127,j=5) rows stay zero from the initial memset.
                    # p=1..126: j=0..5 <- rows 4p-1..4p+4
                    nc.sync.dma_start(
                        out=tCur[1:127, 1 : 1 + NROW * W],
                        in_=AP(cur_t, b * IMG + 3 * W, [[4 * W, 126], [1, NROW * W]]),
                    )
                    # p=0: j=1..5 <- rows 0..4
                    nc.sync.dma_start(
                        out=tCur[0:1, 1 + W : 1 + NROW * W],
                        in_=AP(cur_t, b * IMG, [[4 * W, 1], [1, 5 * W]]),
                    )
                    # p=127: j=0..4 <- rows 507..511
                    nc.sync.dma_start(
                        out=tCur[127:128, 1 : 1 + 5 * W],
                        in_=AP(cur_t, b * IMG + 507 * W, [[4 * W, 1], [1, 5 * W]]),
                    )
                    # --- affinity batches: 3 planes per dma_start.
                    def batch(k0, ring):
                        t = aff_pool.tile([128, 3, NBLK, W], fp32, tag="aff")
                        ring.dma_start(
                            out=t[:],
                            in_=AP(
                                aff_t,
                                (b * 9 + k0) * IMG,
                                [[NBLK * W, 128], [IMG, 3], [W, NBLK], [1, W]],
                            ),
                        )
                        return t

                    B0 = batch(6, nc.scalar)  # planes 6,7,8
                    B1 = batch(0, nc.sync)    # planes 0,1,2
                    B2 = batch(3, nc.scalar)  # planes 3,4,5
                    tC = coa_pool.tile([128, NBLK, W], fp32, tag="coa")
                    nc.sync.dma_start(
                        out=tC[:],
                        in_=coa_d[b, 0].rearrange("(p t) c -> p t c", p=128),
                    )

                    # previous image's store rides after this image's loads
                    flush_store()

                    # PSUM accumulators (2 banks each)
                    ps_lo = ps_pool.tile([128, 2, W], fp32, tag="ps")
                    ps_hi = ps_pool.tile([128, 2, W], fp32, tag="ps")

                    def accum(P, start, stop):
                        for t in range(NBLK):
                            pst = ps_lo[:, t, :] if t < 2 else ps_hi[:, t - 2, :]
                            nc.tensor.matmul(pst, tI[:], P[:, t, :], start=start, stop=stop)

                    # --- products: one mul per tap; dx!=0 taps zero their
                    # bled edge column before the PE reads the product.
                    def edge_fix(eng, P, k):
                        dx = k % 3 - 1
                        if dx == -1:
                            eng.memset(P[:, :, 0:1].bitcast(fp32), 0.0)
                        elif dx == 1:
                            eng.memset(P[:, :, W - 1 : W].bitcast(fp32), 0.0)

                    for i, (tile_, ki, k) in enumerate(
                        [
                            (B0, 0, 6), (B0, 1, 7), (B0, 2, 8),
                            (B1, 0, 0), (B1, 1, 1), (B1, 2, 2),
                        ]
                    ):
                        P = prod_pool.tile([128, NBLK, W], fp32r, tag="prod")
                        nc.vector.tensor_mul(out=P[:], in0=tile_[:, ki], in1=win(tCur, k))
                        edge_fix(nc.vector, P, k)
                        accum(P, start=(i == 0), stop=False)
                    P3 = prod_pool.tile([128, NBLK, W], fp32r, tag="prod")
                    nc.gpsimd.tensor_mul(out=P3[:], in0=B2[:, 0], in1=win(tCur, 3))
                    edge_fix(nc.gpsimd, P3, 3)
                    accum(P3, start=False, stop=False)
                    P5 = prod_pool.tile([128, NBLK, W], fp32r, tag="prod")
                    nc.gpsimd.tensor_mul(out=P5[:], in0=B2[:, 2], in1=win(tCur, 5))
                    edge_fix(nc.gpsimd, P5, 5)
                    accum(P5, start=False, stop=True)

                    # --- root: Osb = aff4*coarse + psum
                    Osb = osb_pool.tile([128, NBLK, W], fp32, tag="osb")
                    nc.gpsimd.tensor_mul(out=Osb[:], in0=B2[:, 1], in1=tC[:])
                    nc.vector.tensor_add(out=Osb[:, 0:2, :], in0=Osb[:, 0:2, :], in1=ps_lo[:])
                    nc.vector.tensor_add(out=Osb[:, 2:4, :], in0=Osb[:, 2:4, :], in1=ps_hi[:])

                    out_rows = out_d[b, 0].rearrange("(p t) c -> p t c", p=128)
                    pending_store[0] = (
                        nc.scalar if b == 0 else nc.sync,
                        out_rows[:],
                        Osb[:],
                    )

            flush_store()

    nc.compile()
    return nc


def _get_program(reps=1):
    global _compiled
    if reps != 1:
        if reps not in _compiled_reps:
            _compiled_reps[reps] = _build_program(reps)
        return _compiled_reps[reps]
    if _compiled is None:
        _compiled = _build_program()
    return _compiled


def _in_maps(affinity, cur_seg, coarse_seg):
    ident = np.eye(128, dtype=np.float32)
    maps = []
    for j in range(N_CORES):
        s = slice(j * B_PER_CORE, (j + 1) * B_PER_CORE)
        maps.append(
            {
                "affinity": np.ascontiguousarray(affinity[s]),
                "cur_seg": np.ascontiguousarray(cur_seg[s]),
                "coarse_seg": np.ascontiguousarray(coarse_seg[s]),
                "ident": ident,
            }
        )
    return maps


_exec_cache = {}


def _build_exec(nc):
    """Sharded PJRT executable for `nc` over 8 cores, mirroring
    bass2jax.run_bass_via_pjrt but staging the FULL input arrays directly
    with a NamedSharding (no 170MB host-side np.concatenate per call)."""
    import jax
    from jax.sharding import Mesh, NamedSharding, PartitionSpec

    try:
        from jax.experimental.shard_map import shard_map
    except ImportError:
        from jax.sharding import shard_map

    import concourse.bass2jax as b2j
    import concourse.mybir as mybir

    b2j.install_neuronx_cc_hook()
    partition_name = nc.partition_id_tensor.name if nc.partition_id_tensor else None
    in_names, out_names, out_avals = [], [], []
    for alloc in nc.m.functions[0].allocations:
        if not isinstance(alloc, mybir.MemoryLocationSet):
            continue
        name = alloc.memorylocations[0].name
        if alloc.kind == "ExternalInput":
            if name != partition_name:
                in_names.append(name)
        elif alloc.kind == "ExternalOutput":
            out_names.append(name)
            out_avals.append(
                jax.core.ShapedArray(tuple(alloc.tensor_shape), mybir.dt.np(alloc.dtype))
            )
    n_params = len(in_names)
    all_in = list(in_names) + list(out_names)
    if partition_name is not None:
        all_in.append(partition_name)

    def _body(*args):
        operands = list(args)
        if partition_name is not None:
            operands.append(b2j.partition_id_tensor())
        return tuple(
            b2j._bass_exec_p.bind(
                *operands,
                out_avals=tuple(out_avals),
                in_names=tuple(all_in),
                out_names=tuple(out_names),
                lowering_input_output_aliases=(),
                sim_require_finite=True,
                sim_require_nnan=True,
                nc=nc,
            )
        )

    devices = jax.devices()[:N_CORES]
    mesh = Mesh(np.asarray(devices), ("core",))
    sharding = NamedSharding(mesh, PartitionSpec("core"))
    n_outs = len(out_names)
    fn = jax.jit(
        shard_map(
            _body,
            mesh=mesh,
            in_specs=(PartitionSpec("core"),) * (n_params + n_outs),
            out_specs=(PartitionSpec("core"),) * n_outs,
            check_rep=False,
        ),
        donate_argnums=tuple(range(n_params, n_params + n_outs)),
        keep_unused=True,
    )
    return fn, in_names, out_names, out_avals, sharding


def kernel(affinity, cur_seg, coarse_seg, i=None, **_unused):
    affinity = np.ascontiguousarray(affinity, dtype=np.float32)
    cur_seg = np.ascontiguousarray(cur_seg, dtype=np.float32)
    coarse_seg = np.ascontiguousarray(coarse_seg, dtype=np.float32)
    nc = _get_program()
    try:
        import jax

        if "fn" not in _exec_cache:
            _exec_cache["fn"] = _build_exec(nc)
        fn, in_names, out_names, out_avals, sharding = _exec_cache["fn"]

        full = {
            "affinity": affinity,
            "cur_seg": cur_seg,
            "coarse_seg": coarse_seg,
            "ident": np.tile(np.eye(128, dtype=np.float32), (N_CORES, 1)),
        }
        dev_in = [jax.device_put(full[name], sharding) for name in in_names]
        zeros = [
            jax.device_put(
                np.zeros((N_CORES * a.shape[0], *a.shape[1:]), a.dtype), sharding
            )
            for a in out_avals
        ]
        outs = fn(*dev_in, *zeros)
        return np.asarray(outs[out_names.index("out")])
    except Exception:
        from concourse.bass_utils import run_bass_kernel_spmd

        res = run_bass_kernel_spmd(
            nc, _in_maps(affinity, cur_seg, coarse_seg), core_ids=list(range(N_CORES))
        )
        return np.concatenate([r["out"] for r in res.results], axis=0)
